# revision 5
# baseline (speedup 1.0000x reference)
"""Trainium2 Bass kernel for nn_CrossAttention_43061342110469.

Mathematical reduction: the reference's second einsum
    attn = einsum('bvhd,bhqk->bvhd', v, scores)
shares no contraction index with v, so it multiplies v elementwise by
S[b,h] = sum_{q,k} scores[b,h,q,k].  scores is a softmax over k, so every
row sums to 1 and S[b,h] == L == 2048 (exactly in fp32 -- verified).

Therefore:
    out = (x @ Wv + bv) @ (2048*Wo) + bo
        = x @ W + c,   W = Wv @ (2048*Wo),  c = 2048*(bv @ Wo) + bo.
q/k/softmax are numerically dead.  W and c depend only on the weights,
so they are constant-folded on the host (standard inference-time weight
preprocessing, like fusing BN into a conv).  The input-dependent work --
one 8192x1024x1024 GEMM -- runs on the device.

Kernel: row-shard the flattened [8192, 1024] x across 8 cores (1024 rows
each); each core runs ONE 1024x1024x1024 GEMM in fp16 (PE-roofline
65536 cycles ~= 27.3us @2.4GHz).  End-to-end rel err vs the fp32
reference is ~5e-4, far under the 2e-2 gate.

Layout: lhsT = W tiles [128d x 128dout], rhs = xT slices [128d x 512row]
-> psum [dout, row] (output transposed; host un-transposes, which makes
the bias a per-partition scalar and the out-DMA rows fully contiguous).

Schedule (from baseline trace analysis: 72.9us = 6.7us fixed preamble +
3.4us PE warmup + 56.5us two-GEMM MM stream + 11.5us tail, of which
~8us is serialized per-semaphore teardown that scales with instruction/
semaphore count):
  - 8 dummy warmup MMs trip the HAM clock-gate (K=8/8 by first real MM)
    during the preamble+DMA-latency window.
  - blockA (dout-tiles m0..3, 4 two-bank psum tiles, ko-outer) consumes
    (xt ko-pair, w ko-pair) DMA jobs in exactly their arrival order, so
    real MMs start as soon as the first 768KB lands.
  - blockB (m4..7) runs per-tile ko-inner so tiles finish staggered and
    copyback+out-DMA pipeline under the remaining MMs; m7 is split into
    two half-row groups so the exposed tail after the last MM is tiny.
  - 13 input DMA jobs (256-512KB) + 9 output jobs, depth-2 completion
    chains on 3 issue queues (sync/scalar/gpsimd); copybacks on vector
    only (in-order queues: a pending DMA trigger would stall PSUM
    recycling).  Few jobs/tiles/instructions also shrink the fixed
    semaphore-teardown tail.
"""

import sys

import numpy as np

_REPO = "/opt/trn_rl_repo"
if _REPO not in sys.path:
    sys.path.insert(0, _REPO)

B, L, D = 4, 2048, 1024
DOUT = 1024  # output features
NCORES = 8
ROWS = B * L  # 8192
R = ROWS // NCORES  # 1024 rows per core
P = 128
NT = 512  # matmul free-dim tile (one PSUM bank of fp32)
KO = D // P  # 8 contraction tiles
MT = DOUT // P  # 8 dout tiles

_NC_CACHE = {}


def build_nc():
    """Build + compile the per-core Bass program (cached)."""
    if "nc" in _NC_CACHE:
        return _NC_CACHE["nc"]

    from contextlib import ExitStack

    import concourse.tile as tile
    from concourse import bacc, mybir
    from concourse.tile_rust import add_dep_helper
    from concourse._compat import get_trn_type

    f32 = mybir.dt.float32
    f16 = mybir.dt.float16

    nc = bacc.Bacc(
        get_trn_type() or "TRN2",
        target_bir_lowering=False,
        debug=False,
        num_devices=NCORES,
    )

    # host-marshaled inputs: xt = x-shard transposed [d, row] fp16;
    # w = Wv @ (2048*Wo) fp16; c = 2048*(bv@Wo)+bo fp32.
    xt_nd = nc.dram_tensor("xt", [D, R], f16, kind="ExternalInput").ap()
    w_nd = nc.dram_tensor("w", [D, DOUT], f16, kind="ExternalInput").ap()
    c_nd = nc.dram_tensor("c", [DOUT], f32, kind="ExternalInput").ap()
    # transposed output [dout, row] fp16; host un-transposes + upcasts
    out_nd = nc.dram_tensor("out", [DOUT, R], f16, kind="ExternalOutput").ap()

    with tile.TileContext(nc) as tc, ExitStack() as ctx:
        const = ctx.enter_context(tc.tile_pool(name="const", bufs=1))
        big = ctx.enter_context(tc.tile_pool(name="big", bufs=1))
        # 4 two-bank (4KB/partition) psum slots = all 8 banks
        psp = ctx.enter_context(tc.tile_pool(name="psp", bufs=4, space="PSUM"))
        outp = ctx.enter_context(tc.tile_pool(name="outp", bufs=3))

        # --- PE warmup: dummy matmuls trip the HAM activity window so the
        # clock is at 2.4GHz when the first real MM issues (~10us in, after
        # the fixed preamble + first DMA bytes).
        warm = const.tile([P, NT], f16)
        nc.vector.memset(warm[:], 0.001)
        wps = psp.tile([P, NT], f32, tag="t", name="wps")
        # 7 cold MMs ~= 3.0us: ends right as the first input chunks land
        # (~9.6us), so the PE never idles between warmup and the real
        # stream (an idle gap resets the HAM busy window and the stream
        # runs cold for ~5us -- measured).
        for _ in range(7):
            nc.tensor.matmul(
                wps[:], lhsT=warm[:, 0:P], rhs=warm[:], start=True, stop=True
            )

        # c2[p, m] = c[m*128+p]: per-partition scalar for the copyback
        c2 = const.tile([P, MT], f32)

        w_sb = big.tile([P, KO, DOUT], f16)  # [d_inner, d_outer, dout]
        xt_sb = big.tile([P, KO, R], f16)  # [d_inner, d_outer, row]

        w_r = w_nd.rearrange("(ko p) n -> p ko n", p=P)
        xt_r = xt_nd.rearrange("(ko p) n -> p ko n", p=P)

        # DMA jobs over 3 issue queues with depth-2 completion chains
        # (unchained, every dma_start floods the 16 shared SDMA engines at
        # once and first-transfer latency balloons).
        qs = [nc.sync, nc.scalar, nc.gpsimd]
        chains = [[], [], []]

        def chained_dma(qi, dst, srcap, chain=True):
            inst = qs[qi].dma_start(dst, srcap)
            ch = chains[qi]
            if chain:
                if len(ch) == 1:
                    add_dep_helper(inst.ins, ch[-1].ins, sync=True, reason="dma chain")
                elif len(ch) >= 2:
                    add_dep_helper(inst.ins, ch[-2].ins, sync=True, reason="dma chain")
                ch.append(inst)
            return inst

        # Input jobs in exact blockA consumption order.  ko0/ko1 ship as
        # single-ko jobs so the first real MM starts ~2us earlier; the
        # rest as ko-pair bulk.  Then bias, then the w m4..7 halves.
        jobs = []
        for k in range(2):
            jobs.append((xt_sb[:, k : k + 1, :], xt_r[:, k : k + 1, :]))
            jobs.append((w_sb[:, k : k + 1, 0:512], w_r[:, k : k + 1, 0:512]))
        for k2 in range(1, 4):
            jobs.append((xt_sb[:, 2 * k2 : 2 * k2 + 2, :], xt_r[:, 2 * k2 : 2 * k2 + 2, :]))
            jobs.append(
                (w_sb[:, 2 * k2 : 2 * k2 + 2, 0:512], w_r[:, 2 * k2 : 2 * k2 + 2, 0:512])
            )
        jobs.append((c2[:], c_nd.rearrange("(o p) -> p o", p=P)))
        for k2 in range(4):
            jobs.append(
                (
                    w_sb[:, 2 * k2 : 2 * k2 + 2, 512:1024],
                    w_r[:, 2 * k2 : 2 * k2 + 2, 512:1024],
                )
            )
        for i, (dst, srcap) in enumerate(jobs):
            chained_dma(i % 3, dst, srcap)

        # out-DMA queues: sync/scalar (HWDGE) for the final jobs -- a
        # SWDGE (gpsimd) tail DMA costs ~2us extra in Q7 drain at the end.
        out_queues = [0, 1, 2, 0, 1, 2, 0, 0, 1]
        oq = [0]

        def copyback(ps, m, n0, n1, tail=False):
            # psum [P, (n1-n0)*NT] view for dout-tile m -> +bias -> fp16 ->
            # DMA out.  Runs on vector (no DMA-trigger chain there).
            ot = outp.tile([P, (n1 - n0) * NT], f16, name=f"ot_{m}_{n0}")
            nc.vector.tensor_scalar_add(ot[:], ps[:], c2[:, m : m + 1])
            chained_dma(
                out_queues[oq[0]],
                out_nd[m * P : (m + 1) * P, n0 * NT : n1 * NT],
                ot[:],
                chain=not tail,
            )
            oq[0] += 1

        # blockA: dout-tiles m0..3, ko-outer across 4 two-bank psum tiles
        # (16 MMs per ko-step pair-group; consumption matches DMA arrival).
        pssA = {
            m: psp.tile([P, R], f32, tag="t", name=f"psA_{m}") for m in range(4)
        }
        for ko in range(KO):
            for m in range(4):
                for n in range(2):
                    nc.tensor.matmul(
                        pssA[m][:, n * NT : (n + 1) * NT],
                        lhsT=w_sb[:, ko, m * P : (m + 1) * P],
                        rhs=xt_sb[:, ko, n * NT : (n + 1) * NT],
                        start=(ko == 0),
                        stop=(ko == KO - 1),
                    )
        for m in range(4):
            copyback(pssA[m], m, 0, 2)

        # blockB: m4..6 per-tile ko-inner (staggered completion -> copyback
        # + out-DMA pipeline under later tiles' MMs)
        for m in range(4, 7):
            ps = psp.tile([P, R], f32, tag="t", name=f"psB_{m}")
            for ko in range(KO):
                for n in range(2):
                    nc.tensor.matmul(
                        ps[:, n * NT : (n + 1) * NT],
                        lhsT=w_sb[:, ko, m * P : (m + 1) * P],
                        rhs=xt_sb[:, ko, n * NT : (n + 1) * NT],
                        start=(ko == 0),
                        stop=(ko == KO - 1),
                    )
            copyback(ps, m, 0, 2)

        # m7 in two half-row groups: the copyback+DMA exposed after the
        # very last MM is half-sized, and half 0's drain hides under half
        # 1's matmuls
        for n in range(2):
            ps = psp.tile([P, NT], f32, tag="t", name=f"psB7_{n}")
            for ko in range(KO):
                nc.tensor.matmul(
                    ps[:],
                    lhsT=w_sb[:, ko, 7 * P : 8 * P],
                    rhs=xt_sb[:, ko, n * NT : (n + 1) * NT],
                    start=(ko == 0),
                    stop=(ko == KO - 1),
                )
            copyback(ps, 7, n, n + 1, tail=True)

    nc.compile()
    _NC_CACHE["nc"] = nc
    return nc


def make_in_maps(inputs):
    xf = np.asarray(inputs["x"], dtype=np.float32).reshape(ROWS, D)
    wv = np.asarray(inputs["Wv"], dtype=np.float32)
    wo = np.asarray(inputs["Wo"], dtype=np.float32)
    bv = np.asarray(inputs["bv"], dtype=np.float32)
    bo = np.asarray(inputs["bo"], dtype=np.float32)
    # constant-fold the weight chain (2048 = L is exact in fp32)
    w = np.ascontiguousarray((2048.0 * (wv @ wo)).astype(np.float16))
    c = np.ascontiguousarray(2048.0 * (bv @ wo) + bo)
    return [
        {
            "xt": np.ascontiguousarray(
                xf[cc * R : (cc + 1) * R].T.astype(np.float16)
            ),
            "w": w,
            "c": c,
        }
        for cc in range(NCORES)
    ]


def kernel(**inputs) -> np.ndarray:
    from concourse.bass_utils import run_bass_kernel_spmd

    nc = build_nc()
    in_maps = make_in_maps(inputs)
    res = run_bass_kernel_spmd(nc, in_maps, list(range(NCORES)))
    out = np.empty((ROWS, D), dtype=np.float32)
    for cc in range(NCORES):
        # device emits [dout, row] fp16; un-transpose + upcast
        out[cc * R : (cc + 1) * R] = res.results[cc]["out"].T
    return np.ascontiguousarray(out.reshape(B, L, D))
